# revision 2
# baseline (speedup 1.0000x reference)
"""Clustered-attention Trainium2 kernel (Bass/Tile), 8-core SPMD.

Problem (per batch b, variable k, with L=512, V=32, D=64, C=8 clusters):
    S      = sum_v key[b,:,v,:]                       # (L, D) shared key-sum
    sc     = query[b,:,k,:] @ S.T / sqrt(D)           # (L, L)
    sc     = where(label[i]==label[j], sc, -inf)
    A      = softmax(sc, axis=-1)
    out    = A @ value[b,:,k,:]

Sharding: 8 cores = 4 batches x 2 halves of the v axis (16 heads/core).

Device algorithm per core (all FLOPs on device):
  - keysum S via DVE reduction over v, PE-transposed into S^T.
  - The cluster mask is folded into the scores matmul: the contraction dim
    is extended by 8 one-hot label rows scaled by 8*B (B=96) on the lhsT
    side and 1.0 on the rhs side, so z = q.s + 8B*[same cluster]; the exp
    activation computes exp(z/8 - B), which is exp(q.s/8) for same-cluster
    pairs and <= e^-61 (vs real terms >= e^-35) otherwise -- i.e. an exact
    -inf mask up to ~1e-10 relative.
  - scores^T chunks [128j, 512i] on PE (fp16 operands, fp32 accumulate),
    exp on ScalarE (bf16 out), then U = [V|1]^T-style matmul accumulating
    E^T chunks as lhsT so the output lands directly as [i, d|denom] in
    PSUM; one reciprocal + 4 broadcast multiplies per head normalize it.
"""

import numpy as np

import concourse.bass as bass
import concourse.tile as tile
from concourse import mybir
from concourse.masks import make_identity
from concourse.tile import TileContext, ScopedClock

B, L, V, D = 4, 512, 32, 64
NC = 8  # cores
VH = V // 2  # heads (variables) per core
NJ = L // 128  # j/i chunks
BIAS = 96.0  # mask bias (see module docstring)
F32 = mybir.dt.float32
F16 = mybir.dt.float16
BF16 = mybir.dt.bfloat16

PROFILE = False  # set True from a harness to enable NTFF tracing
LAST_RESULT = None  # BassKernelResults of the most recent run

_PATCHED = False


def _patch_tile_drain():
    """Walrus on this image rejects multiple sync-waits on one instruction
    ("Too many sync wait commands"). Legalize by splitting surplus waits
    onto NoOp instructions inserted just before, on the same engine —
    identical semantics (the engine stalls at each wait in order)."""
    global _PATCHED
    if _PATCHED:
        return
    _PATCHED = True

    _orig_add = TileContext._add_instruction

    def _add_instruction(self, inst):
        si = getattr(inst, "sync_info", None)
        if (
            si is not None
            and si.on_wait
            and len(si.on_wait) > 1
            and inst.engine != mybir.EngineType.Unassigned
        ):
            waits = list(si.on_wait)
            for w in waits[:-1]:
                nop = mybir.InstNoOp(name=self.nc.get_next_instruction_name())
                nop.engine = inst.engine
                nop.sync_info = mybir.SyncInfo(on_wait=[w], on_update=[])
                _orig_add(self, nop)
            inst.sync_info = mybir.SyncInfo(
                on_wait=[waits[-1]], on_update=list(si.on_update or [])
            )
        _orig_add(self, inst)

    TileContext._add_instruction = _add_instruction

    def _drain_and_barrier(self, tick_clock, wait_clock):
        nc = self.nc
        drain_inst = nc.sync.drain()
        wait_clock.add_sem_waits(
            drain_inst.ins, ScopedClock({None: tick_clock.global_clock})
        )
        si = drain_inst.ins.sync_info
        maxw = 1
        if si is not None and si.on_wait and len(si.on_wait) > maxw:
            waits = list(si.on_wait)
            drain_inst.ins.sync_info = mybir.SyncInfo(
                on_wait=waits[:maxw], on_update=list(si.on_update or [])
            )
            for i in range(maxw, len(waits), maxw):
                nop = nc.sync.nop(nofuse=True, hint=f"drain_split_{i}")
                nop.ins.sync_info = mybir.SyncInfo(
                    on_wait=waits[i : i + maxw], on_update=[]
                )
        nc.all_engine_barrier()
        assert self.sems is not None
        popped = nc._tile_sem_poison_stack.pop()
        assert popped is self._sem_poison
        nc.clear_and_free_semaphores(list(self.sems.allocated().values()))
        nc.all_engine_barrier()

    TileContext._drain_and_barrier = _drain_and_barrier


def _build_nc():
    nc = bass.Bass("TRN2", target_bir_lowering=False, debug=False)

    q_t = nc.dram_tensor("q_t", [VH, D, L], F32, kind="ExternalInput").ap()
    k_in = nc.dram_tensor("k", [L, V, D], F32, kind="ExternalInput").ap()
    v_in = nc.dram_tensor("v", [L, VH, D], F32, kind="ExternalInput").ap()
    lab = nc.dram_tensor("lab", [1, L], F32, kind="ExternalInput").ap()
    iota8 = nc.dram_tensor("iota8", [8, 1], F32, kind="ExternalInput").ap()
    o_out = nc.dram_tensor("o", [L, VH, D], F32, kind="ExternalOutput").ap()

    with TileContext(nc) as tc:
        with (
            tc.tile_pool(name="singles", bufs=1) as singles,
            tc.tile_pool(name="kpool", bufs=2) as kpool,
            tc.tile_pool(name="vstage", bufs=2) as vstage,
            tc.tile_pool(name="qstage", bufs=3) as qstage,
            tc.tile_pool(name="qtb", bufs=3) as qtbpool,
            tc.tile_pool(name="epool", bufs=8) as epool,
            tc.tile_pool(name="rpool", bufs=3) as rpool,
            tc.tile_pool(name="ps_score", bufs=4, space="PSUM") as ps_score,
            tc.tile_pool(name="ps_u", bufs=2, space="PSUM") as ps_u,
            tc.tile_pool(name="ps_t", bufs=2, space="PSUM") as ps_t,
        ):
            # ---- constants ----
            identity = singles.tile([128, 128], F32)
            make_identity(nc, identity)
            negb = singles.tile([128, 1], F32)
            nc.vector.memset(negb, -BIAS)

            # ---- one-hot label rows ----
            lab_sb = singles.tile([8, L], F32)
            lab_bcast = bass.AP(tensor=lab.tensor, offset=lab.offset,
                                ap=[[0, 8]] + list(lab.ap[1:]))
            nc.gpsimd.dma_start(out=lab_sb, in_=lab_bcast)
            iota_sb = singles.tile([8, 1], F32)
            nc.gpsimd.dma_start(out=iota_sb, in_=iota8)
            onehot = singles.tile([8, L], F32)
            nc.vector.tensor_scalar(onehot, lab_sb, iota_sb, None,
                                    op0=mybir.AluOpType.is_equal)
            # rhs-side rows (1.0 where label==c) and lhsT-side rows (*8B)
            oh16 = singles.tile([8, L], F16)
            nc.vector.tensor_copy(oh16, onehot)
            oh768 = singles.tile([8, L], F16)
            nc.vector.tensor_scalar_mul(oh768, onehot, 8.0 * BIAS)

            # ---- keysum -> S^T (fp16), plus one-hot rows -> STB ----
            stb = singles.tile([D + 8, L], F16)
            for jc in range(NJ):
                kc = kpool.tile([128, V * D], F32, tag="kc")
                nc.sync.dma_start(
                    out=kc,
                    in_=k_in[jc * 128 : (jc + 1) * 128].rearrange("p v d -> p (v d)"),
                )
                s_chunk = kpool.tile([128, D], F32, tag="schunk")
                nc.vector.reduce_sum(
                    out=s_chunk,
                    in_=kc.rearrange("p (v d) -> p d v", v=V, d=D),
                    axis=mybir.AxisListType.X,
                )
                st_ps = ps_t.tile([D, 128], F32)
                nc.tensor.transpose(st_ps, s_chunk, identity)
                nc.vector.tensor_copy(stb[0:D, jc * 128 : (jc + 1) * 128], st_ps)
            nc.sync.dma_start(out=stb[D : D + 8, :], in_=oh768)

            # ---- value chunks -> bf16 with ones column ----
            vcast = singles.tile([128, NJ, VH, D + 2], BF16)
            for jc in range(NJ):
                vc_f32 = vstage.tile([128, VH * D], F32, tag="vc")
                nc.sync.dma_start(
                    out=vc_f32,
                    in_=v_in[jc * 128 : (jc + 1) * 128].rearrange("p h d -> p (h d)"),
                )
                nc.vector.tensor_copy(
                    vcast[:, jc, :, 0:D],
                    vc_f32.rearrange("p (h d) -> p h d", h=VH, d=D),
                )
                nc.vector.memset(vcast[:, jc, :, D : D + 1], 1.0)

            # ---- output collect tile ----
            oc = singles.tile([128, NJ, VH, D], F32)

            # ---- per-head pipeline ----
            for h in range(VH):
                qs = qstage.tile([D, L], F32, tag="qs")
                nc.sync.dma_start(out=qs, in_=q_t[h])
                qtb = qtbpool.tile([D + 8, L], F16, tag="qtb")
                nc.vector.tensor_copy(qtb[0:D, :], qs)
                nc.sync.dma_start(out=qtb[D : D + 8, :], in_=oh16)

                e_tiles = []
                for jc in range(NJ):
                    ps = ps_score.tile([128, L], F32, tag="ps")
                    nc.tensor.matmul(
                        ps, lhsT=stb[:, jc * 128 : (jc + 1) * 128], rhs=qtb,
                        start=True, stop=True,
                    )
                    e_t = epool.tile([128, L], BF16, tag="et")
                    nc.scalar.activation(
                        e_t, ps, mybir.ActivationFunctionType.Exp,
                        bias=negb, scale=1.0 / 8.0,
                    )
                    e_tiles.append(e_t)

                psu = ps_u.tile([128, NJ, D + 1], F32)
                for si in range(NJ):
                    for jc in range(NJ):
                        nc.tensor.matmul(
                            psu[:, si, :],
                            lhsT=e_tiles[jc][:, si * 128 : (si + 1) * 128],
                            rhs=vcast[:, jc, h, 0 : D + 1],
                            start=(jc == 0), stop=(jc == NJ - 1),
                        )
                rinv = rpool.tile([128, NJ], F32, tag="rinv")
                nc.vector.reciprocal(rinv, psu[:, :, D])
                for si in range(NJ):
                    nc.vector.tensor_scalar_mul(
                        oc[:, si, h, :], psu[:, si, 0:D], rinv[:, si : si + 1]
                    )

            # ---- write output ----
            for si in range(NJ):
                nc.sync.dma_start(
                    out=o_out[si * 128 : (si + 1) * 128], in_=oc[:, si]
                )
    return nc


_NC_CACHE = None


def _get_nc():
    global _NC_CACHE
    if _NC_CACHE is None:
        _patch_tile_drain()
        _NC_CACHE = _build_nc()
    return _NC_CACHE


def kernel(query, key, value, label_arr):
    """Full inputs (B,L,V,D)/(B,L) -> full output (B,L,V,D)."""
    global LAST_RESULT
    from concourse.bass_utils import run_bass_kernel_spmd

    query = np.asarray(query, dtype=np.float32)
    key = np.asarray(key, dtype=np.float32)
    value = np.asarray(value, dtype=np.float32)
    lab_f32 = np.asarray(label_arr).astype(np.float32)
    iota = np.arange(8, dtype=np.float32).reshape(8, 1)

    in_maps = []
    for c in range(NC):
        b, v0 = c // 2, (c % 2) * VH
        in_maps.append({
            "q_t": np.ascontiguousarray(
                query[b, :, v0 : v0 + VH, :].transpose(1, 2, 0)
            ),
            "k": np.ascontiguousarray(key[b]),
            "v": np.ascontiguousarray(value[b, :, v0 : v0 + VH, :]),
            "lab": lab_f32[b].reshape(1, L).copy(),
            "iota8": iota,
        })

    nc = _get_nc()
    kwargs = {}
    if PROFILE:
        kwargs["trace"] = True
    res = run_bass_kernel_spmd(nc, in_maps, list(range(NC)), **kwargs)
    LAST_RESULT = res

    out = np.empty((B, L, V, D), dtype=np.float32)
    for c in range(NC):
        b, v0 = c // 2, (c % 2) * VH
        out[b, :, v0 : v0 + VH, :] = res.results[c]["o"]
    return out
